# revision 36
# baseline (speedup 1.0000x reference)
"""MoH (Mixture-of-Heads) attention kernel for Trainium2, 8 NeuronCores.

Strategy: data-parallel over batch (32 batches -> 4 per core), weights
replicated, no collectives.  Matmuls in bf16 (fp32 PSUM accumulation)
except the q projection, which runs in fp8e4 with DoubleRow perf mode
(2x128 contraction per instruction -> ~2x faster streaming).  fp8 on the
k side too would push the score noise over the 2e-2 budget (measured
2.33e-2 both / 1.83e-2 q-only / 0.41e-2 bf16), and v/out errors flow
linearly to the output, so those stay bf16.

Layouts (per core):
  - host pre-transposes q/k/v to [B, D, S] and casts to bf16, so the
    projections need no on-device transposes:
       qpT[d',s] = sum_d Wq[d,d'] qT[d,s]   (lhsT = Wq as stored)
       vp[s,d']  = sum_d vT[d,s] Wv[d,d']   (lhsT = vT)
  - heads live in the partition dim of qpT/kpT (64 rows each), so
    transposed scores ST[k,q] = kh @ qh^T come straight from matmuls of
    qpT/kpT slices; softmax runs without max-subtraction (scores are
    O(1)); fully-masked k-blocks are skipped.
  - causal masking: post-exp multiply of each diagonal 128x128 block by
    a lower-triangular 0/1 bf16 tile built on-device with affine_select
    (general masks fall back to additive -1e9 tiles pre-exp).
  - the reference's "scores row 0 := 0" rule becomes "STexp[:,0] := 1"
    plus ones-matmuls for the skipped k-blocks.
  - vp carries a ones-column so the attention matmul also produces the
    softmax denominator (row 64 of the [65, q] PSUM tile).
  - routing: gates = softmax(qpT^T @ Wg), hard top-2 of 12 via two
    reduce_max passes, mean over S via a ones-vector matmul; the
    per-(batch,head) routing scalar and 1/denominator are fused into the
    single DVE op that moves ctx^T from PSUM to SBUF.
  - out = ctxT^T @ Wo from the same partition-sliced ctxT tiles.

Scheduling: batches flow through a software pipeline.  During batch i's
16-head attention loop (ACT-bound: one exp per score pack), the PE-heavy
projection chunks of batch i+1 and the output-projection chunks of batch
i-1 are issue-interleaved so the Tensor engine never starves; within the
head loop, head h+1's score matmuls issue before head h's ctx matmuls so
the exp latency hides behind PE work.  Input and output DMAs issue from
SP, early weights from ACT, late weights from GPSIMD, keeping the
compute engines' queues clean at startup.
"""

import sys

_TRN_REPO = "/opt/trn_rl_repo"
if _TRN_REPO not in sys.path:
    sys.path.insert(0, _TRN_REPO)

import numpy as np
import ml_dtypes

B, S, D = 32, 512, 1024
H, DK = 16, 64
H_SH, K_SEL = 4, 2
H_DYN = H - H_SH
N_CORES = 8
B_LOC = B // N_CORES
SB = S // 128      # 4 s-blocks
DT = D // 128      # 8 d-tiles
NEG = -1e9

_CACHE = {}
PROFILE = False          # set by test harness to capture an NTFF trace
LAST = {}                # exec_time_ns / profile path from the last run


def _classify_mask(mask):
    """Host-side: derive block structure from the [S,S] 0/1 mask.

    Returns (qs, mixed, uniq_tiles, causal) where
      qs[kb]    = first q (multiple of 128) kept for k-block kb, or None
      mixed[(qb,kb)] = index into uniq_tiles for blocks needing an
                  additive mask tile (maskT layout [k_local, q_local]),
                  or -1 for all-masked blocks inside the computed range
      uniq_tiles = list of [128,128] f32 additive tiles
      causal    = True if mask is exactly lower-triangular (fast path:
                  post-exp multiplicative triangular masking)
    """
    m = mask.astype(bool)
    if not m[1:].any(axis=1).all():
        raise NotImplementedError(
            "a query row (>0) is fully masked; uniform-softmax fallback "
            "for fully-masked rows is not implemented"
        )
    causal = bool(np.array_equal(m, np.tril(np.ones_like(m))))
    qs = []
    mixed = {}
    uniq = []
    uniq_key = {}
    for kb in range(SB):
        first = None
        for qb in range(SB):
            blk = m[qb * 128:(qb + 1) * 128, kb * 128:(kb + 1) * 128]
            if blk.any():
                if first is None:
                    first = qb * 128
                if not blk.all():
                    add = np.where(blk.T, 0.0, np.float32(NEG)).astype(np.float32)
                    key = add.tobytes()
                    if key not in uniq_key:
                        uniq_key[key] = len(uniq)
                        uniq.append(add)
                    mixed[(qb, kb)] = uniq_key[key]
            elif first is not None:
                mixed[(qb, kb)] = -1
        qs.append(first)
    return qs, mixed, uniq, causal


def _build(qs, mixed, uniq_n, causal, b_loc=B_LOC, has_bvo=False, repeat=1):
    import os
    PACK = int(os.environ.get("K_PACK", "1"))
    MM_BUFS = int(os.environ.get("K_MM_BUFS", "4"))
    CTX_BUFS = int(os.environ.get("K_CTX_BUFS", "2"))
    STEXP_BUFS = int(os.environ.get("K_STEXP_BUFS", "6"))
    IN_BUFS = int(os.environ.get("K_IN_BUFS", "2"))
    TRIMASK_POOL = int(os.environ.get("K_TRIMASK_POOL", "0"))
    HPIPE = int(os.environ.get("K_HPIPE", "1"))
    # K_ONES64=1 replicates the denominator across PSUM partitions 64..127
    # via 64 ones-columns in vp, removing the per-head partition_broadcast.
    # Measured SLOWER on hardware (+138us/iter): the wider stationary tile
    # makes every ctx ldweights load 128 columns instead of 65 -- weight
    # loads are not free on HW even though the cost model says they are.
    ONES64 = int(os.environ.get("K_ONES64", "0"))
    VW = 128 if ONES64 else DK + 1    # vp row width per head
    import concourse.bacc as bacc
    import concourse.tile as tile
    import concourse.mybir as mybir
    from contextlib import ExitStack

    f32 = mybir.dt.float32
    bf16 = mybir.dt.bfloat16
    fp8 = mybir.dt.float8e4
    DRow = mybir.MatmulPerfMode.DoubleRow
    AF = mybir.ActivationFunctionType
    ALU = mybir.AluOpType

    nc = bacc.Bacc(trn_type="TRN2", target_bir_lowering=False, debug=False)

    # q input + weight in fp8e4 (weight pre-scaled by 64 on host; the 1/64
    # is folded into the projection-evacuation scale).  DoubleRow matmuls
    # contract 2x128 per instruction -> half the PE streaming time.  The
    # k side stays bf16: fp8 on both sides of the score product doubles
    # the score noise and breaches the 2e-2 budget (measured 2.33e-2 vs
    # 1.82e-2 with q only).
    qT = nc.dram_tensor("qT8", [b_loc, D, S], fp8, kind="ExternalInput").ap()
    kT = nc.dram_tensor("kT", [b_loc, D, S], bf16, kind="ExternalInput").ap()
    vT = nc.dram_tensor("vT", [b_loc, D, S], bf16, kind="ExternalInput").ap()
    wq = nc.dram_tensor("wq8", [D // 2, 2, D], fp8, kind="ExternalInput").ap()
    wk = nc.dram_tensor("wk", [D, D], bf16, kind="ExternalInput").ap()
    wv = nc.dram_tensor("wv", [D, D], bf16, kind="ExternalInput").ap()
    wo = nc.dram_tensor("wo", [D, D], bf16, kind="ExternalInput").ap()
    wg = nc.dram_tensor("wg", [D, H_DYN], bf16, kind="ExternalInput").ap()
    bqt = nc.dram_tensor("bqt", [128, DT], f32, kind="ExternalInput").ap()
    bkt = nc.dram_tensor("bkt", [128, DT], f32, kind="ExternalInput").ap()
    if has_bvo:
        bvb = nc.dram_tensor("bvb", [1, D], f32, kind="ExternalInput").ap()
        bob = nc.dram_tensor("bob", [1, D], f32, kind="ExternalInput").ap()
    if uniq_n:
        maskt = nc.dram_tensor(
            "maskt", [uniq_n, 128, 128], f32, kind="ExternalInput"
        ).ap()
    out = nc.dram_tensor("out", [b_loc, S, D], f32, kind="ExternalOutput").ap()

    # pack score k-blocks into shared PSUM tiles (<=512 f32 per bank)
    packs = []   # list of list[(kb, q0, n, off)]
    if PACK:
        cur, used = [], 0
        for kb in range(SB):
            if qs[kb] is None:
                continue
            n = S - qs[kb]
            if used + n > 512 and cur:
                packs.append(cur)
                cur, used = [], 0
            cur.append((kb, qs[kb], n, used))
            used += n
        if cur:
            packs.append(cur)
    else:
        for kb in range(SB):
            if qs[kb] is None:
                continue
            packs.append([(kb, qs[kb], S - qs[kb], 0)])

    with tile.TileContext(nc) as tc, ExitStack() as ctx:
        const = ctx.enter_context(tc.tile_pool(name="const", bufs=1))
        act = ctx.enter_context(tc.tile_pool(name="act", bufs=2))
        small = ctx.enter_context(tc.tile_pool(name="small", bufs=2))
        psum = ctx.enter_context(tc.tile_pool(name="psum", bufs=1, space="PSUM"))

        states = {}

        class St:
            pass

        def load(i, b):
            st = states.setdefault(i, St())
            st.b = b
            st.ins = {}
            for nm, ap, dt8 in (("q", qT, True), ("k", kT, False),
                                ("v", vT, False)):
                t = act.tile([128, DT, S], fp8 if dt8 else bf16,
                             name=f"in_{nm}", tag=f"in_{nm}", bufs=IN_BUFS)
                for d in range(DT):
                    nc.sync.dma_start(t[:, d, :], ap[b, d * 128:(d + 1) * 128, :])
                st.ins[nm] = t
            return st

        # ---- constants: wq + biases on ACT's queue (needed first), the
        # rest on GPSIMD's queue so ACT can start qpT evacuation early ----
        ones_bf = const.tile([128, 1], bf16, name="ones_bf", tag="ones_bf")
        nc.vector.memset(ones_bf[:], 1.0)
        ones_f32 = const.tile([128, 1], f32, name="ones_f32", tag="ones_f32")
        nc.vector.memset(ones_f32[:], 1.0)
        # lower-triangular (k <= q) 0/1 multiplicative mask in [k, q] layout
        trib = const.tile([128, 128], bf16, name="trib", tag="trib")
        if causal:
            nc.gpsimd.memset(trib[:], 1.0)
            nc.gpsimd.affine_select(
                out=trib[:], in_=trib[:],
                compare_op=mybir.AluOpType.is_ge, fill=0.0,
                base=0, pattern=[[1, 128]], channel_multiplier=-1,
            )

        bq_sb = const.tile([128, DT], f32, name="bq_sb", tag="bq_sb")
        nc.scalar.dma_start(bq_sb[:], bqt[:])
        bk_sb = const.tile([128, DT], f32, name="bk_sb", tag="bk_sb")
        nc.scalar.dma_start(bk_sb[:], bkt[:])
        w_tiles = {}
        for t2 in range(DT // 2):
            t = const.tile([128, 2, D], fp8, name=f"wq8{t2}", tag=f"wq8{t2}")
            nc.scalar.dma_start(t[:], wq[t2 * 128:(t2 + 1) * 128, :, :])
            w_tiles.setdefault("wq8", []).append(t)
        for wname, wap, eng in (("wk", wk, nc.gpsimd), ("wv", wv, nc.gpsimd),
                                ("wo", wo, nc.gpsimd)):
            tl = []
            for d in range(DT):
                t = const.tile([128, D], bf16, name=f"{wname}{d}", tag=f"{wname}{d}")
                eng.dma_start(t[:], wap[d * 128:(d + 1) * 128, :])
                tl.append(t)
            w_tiles[wname] = tl
        wg_tiles = []
        for d in range(DT):
            t = const.tile([128, H_DYN], bf16, name=f"wg{d}", tag=f"wg{d}")
            nc.gpsimd.dma_start(t[:], wg[d * 128:(d + 1) * 128, :])
            wg_tiles.append(t)
        if has_bvo:
            bv_sb = const.tile([1, D], f32, name="bv_sb", tag="bv_sb")
            nc.gpsimd.dma_start(bv_sb[:], bvb[:])
            bo_sb = const.tile([1, D], f32, name="bo_sb", tag="bo_sb")
            nc.gpsimd.dma_start(bo_sb[:], bob[:])
            bvb_sb = const.tile([128, D], f32, name="bvb_sb", tag="bvb_sb")
            nc.gpsimd.partition_broadcast(bvb_sb[:], bv_sb[:])
            bob_sb = const.tile([128, D], f32, name="bob_sb", tag="bob_sb")
            nc.gpsimd.partition_broadcast(bob_sb[:], bo_sb[:])

        mask_tiles = []
        for u in range(uniq_n):
            t = const.tile([128, 128], f32, name=f"mask{u}", tag=f"mask{u}")
            if not causal:
                nc.gpsimd.dma_start(t[:], maskt[u])
            mask_tiles.append(t)

        # ---------------------------------------------------------------
        def proj_chunks(i):
            """Allocate batch i's projection outputs; return issue closures."""
            st = states[i]
            st.qpT = act.tile([128, DT, S], bf16, name="qpT", tag="qpT")
            st.kpT = act.tile([128, DT, S], bf16, name="kpT", tag="kpT")
            st.vp = act.tile([128, SB, H, VW], bf16, name="vp", tag="vp")
            chunks = []

            def q_chunk(t):
                def go():
                    ps = psum.tile([128, S], f32, name="mm_ps", tag="mm",
                                   bufs=MM_BUFS)
                    src4 = st.ins["q"][:].rearrange("p (t2 i) s -> p t2 i s", i=2)
                    for t2 in range(DT // 2):
                        nc.tensor.matmul(
                            ps[:],
                            w_tiles["wq8"][t2][:, :, t * 128:(t + 1) * 128],
                            src4[:, t2, :, :],
                            start=(t2 == 0),
                            stop=(t2 == DT // 2 - 1),
                            perf_mode=DRow,
                        )
                    nc.scalar.activation(
                        st.qpT[:, t, :], ps[:], AF.Identity,
                        bias=bq_sb[:, t:t + 1], scale=1.0 / 64,
                    )
                return go

            def k_chunk(t):
                def go():
                    ps = psum.tile([128, S], f32, name="mm_ps", tag="mm",
                                   bufs=MM_BUFS)
                    for d in range(DT):
                        nc.tensor.matmul(
                            ps[:],
                            w_tiles["wk"][d][:, t * 128:(t + 1) * 128],
                            st.ins["k"][:, d, :],
                            start=(d == 0),
                            stop=(d == DT - 1),
                        )
                    nc.vector.tensor_scalar_add(
                        st.kpT[:, t, :], ps[:], bk_sb[:, t:t + 1]
                    )
                return go

            for t in range(DT):
                chunks.append(q_chunk(t))
            for t in range(DT):
                chunks.append(k_chunk(t))

            def v_chunk(sb, c):
                def go():
                    if sb == 0 and c == 0 and (not ONES64 or i < 2):
                        # ONES64: the ones region is never overwritten, and
                        # the vp tag rotates 2 buffers, so batches 0/1 cover
                        # all buffers and later batches skip the memset
                        nc.vector.memset(st.vp[:, :, :, DK:VW], 1.0)
                    ps = psum.tile([128, S], f32, name="mmv_ps", tag="mm",
                                   bufs=MM_BUFS)
                    for d in range(DT):
                        nc.tensor.matmul(
                            ps[:],
                            st.ins["v"][:, d, sb * 128:(sb + 1) * 128],
                            w_tiles["wv"][d][:, c * 512:(c + 1) * 512],
                            start=(d == 0),
                            stop=(d == DT - 1),
                        )
                    src2 = ps[:].rearrange("p (h e) -> p h e", e=DK)
                    dst2 = st.vp[:, sb, c * 8:(c + 1) * 8, 0:DK]
                    if has_bvo:
                        nc.vector.scalar_tensor_tensor(
                            dst2, src2, 1.0,
                            bvb_sb[:, c * 512:(c + 1) * 512].rearrange(
                                "p (h e) -> p h e", e=DK),
                            op0=ALU.mult, op1=ALU.add,
                        )
                    else:
                        nc.vector.tensor_copy(dst2, src2)
                return go

            for sb in range(SB):
                for c in range(2):
                    chunks.append(v_chunk(sb, c))
            return chunks

        def routing(i):
            st = states[i]
            qpT = st.qpT
            ps_r = psum.tile([1, H_DYN], f32, name="ps_r", tag="rsum", bufs=1)
            for sb in range(SB):
                ps_g = psum.tile([128, H_DYN], f32, name="ps_g", tag="gat", bufs=1)
                for t in range(DT):
                    nc.tensor.matmul(
                        ps_g[:],
                        qpT[:, t, sb * 128:(sb + 1) * 128],
                        wg_tiles[t][:],
                        start=(t == 0),
                        stop=(t == DT - 1),
                    )
                gexp = small.tile([128, H_DYN], f32, name="gexp", tag="gexp")
                gsum = small.tile([128, 1], f32, name="gsum", tag="gsum")
                nc.scalar.activation(gexp[:], ps_g[:], AF.Exp, accum_out=gsum[:])
                ginv = small.tile([128, 1], f32, name="ginv", tag="ginv")
                nc.vector.reciprocal(ginv[:], gsum[:])
                m1 = small.tile([128, 1], f32, name="m1", tag="m1")
                nc.vector.reduce_max(m1[:], gexp[:], axis=mybir.AxisListType.X)
                eqm = small.tile([128, H_DYN], f32, name="eqm", tag="eqm")
                nc.vector.tensor_scalar(eqm[:], gexp[:], m1[:], None,
                                        op0=ALU.is_equal)
                g2 = small.tile([128, H_DYN], f32, name="g2", tag="g2")
                nc.vector.scalar_tensor_tensor(
                    g2[:], eqm[:], NEG, gexp[:], op0=ALU.mult, op1=ALU.add
                )
                m2 = small.tile([128, 1], f32, name="m2", tag="m2")
                nc.vector.reduce_max(m2[:], g2[:], axis=mybir.AxisListType.X)
                sel = small.tile([128, H_DYN], f32, name="sel", tag="sel")
                nc.vector.tensor_scalar(sel[:], gexp[:], m2[:], None,
                                        op0=ALU.is_ge)
                dyn = small.tile([128, H_DYN], f32, name="dyn", tag="dyn")
                nc.vector.tensor_scalar(dyn[:], gexp[:], ginv[:], None,
                                        op0=ALU.mult)
                nc.vector.tensor_tensor(dyn[:], dyn[:], sel[:], op=ALU.mult)
                nc.tensor.matmul(
                    ps_r[:], ones_f32[:], dyn[:],
                    start=(sb == 0), stop=(sb == SB - 1),
                    skip_group_check=True,
                )
            routing_sb = small.tile([1, H], f32, name="routing_sb", tag="routing_sb")
            nc.vector.memset(routing_sb[0:1, 0:H_SH], 1.0)
            nc.scalar.mul(routing_sb[0:1, H_SH:H], ps_r[0:1, :], 1.0 / S)
            routing_bc = small.tile([128, H], f32, name="routing_bc", tag="routing_bc")
            nc.gpsimd.partition_broadcast(routing_bc[:], routing_sb[:])
            st.routing_bc = routing_bc

        def attn_scores(st, h):
            """Score matmuls + exp + masking for head h; returns the se
            slices the ctx phase needs (issued separately so the next
            head's scores hide this head's exp latency on the PE)."""
            qpT, kpT = st.qpT, st.kpT
            vp = st.vp
            ph = (h % 2) * 64
            th = h // 2
            ctx_mms = []          # deferred (lhsT, rhs, (q0, n))
            for grp in packs:
                tot = grp[-1][3] + grp[-1][2]
                ps_st = psum.tile([128, 512], f32, name="ps_st", tag="mm", bufs=MM_BUFS)
                se = small.tile([128, 512], bf16, name="stexp", tag="stexp", bufs=STEXP_BUFS)
                for (kb, q0, n, off) in grp:
                    nc.tensor.matmul(
                        ps_st[:, off:off + n],
                        kpT[ph:ph + 64, th, kb * 128:(kb + 1) * 128],
                        qpT[ph:ph + 64, th, q0:S],
                        start=True, stop=True,
                        skip_group_check=True,
                    )
                    if not causal:
                        for qb in range(q0 // 128, SB):
                            mi = mixed.get((qb, kb))
                            if mi is not None and mi >= 0:
                                sl = ps_st[:, off + qb * 128 - q0:
                                           off + (qb + 1) * 128 - q0]
                                nc.vector.tensor_tensor(
                                    sl, sl, mask_tiles[mi][:], op=ALU.add
                                )
                nc.scalar.activation(
                    se[:, 0:tot], ps_st[:, 0:tot], AF.Exp, scale=1.0 / np.sqrt(DK)
                )
                if causal:
                    for (kb, q0, n, off) in grp:
                        if TRIMASK_POOL:
                            nc.gpsimd.affine_select(
                                out=se[:, off:off + 128],
                                in_=se[:, off:off + 128],
                                compare_op=ALU.is_ge, fill=0.0,
                                base=0, pattern=[[1, 128]],
                                channel_multiplier=-1,
                            )
                        else:
                            nc.vector.tensor_tensor(
                                se[:, off:off + 128], se[:, off:off + 128],
                                trib[:], op=ALU.mult,
                            )
                        if q0 == 0:
                            nc.vector.memset(se[:, off:off + 1], 1.0)
                else:
                    for (kb, q0, n, off) in grp:
                        for qb in range(q0 // 128, SB):
                            if mixed.get((qb, kb)) == -1:
                                nc.vector.memset(
                                    se[:, off + qb * 128 - q0:
                                       off + (qb + 1) * 128 - q0], 0.0)
                        if q0 == 0:
                            nc.vector.memset(se[:, off:off + 1], 1.0)
                for (kb, q0, n, off) in grp:
                    ctx_mms.append((vp[:, kb, h, :], se[:, off:off + n],
                                    (q0, n)))
            return ctx_mms

        def attn_ctx(st, h, ctx_mms):
            vp, ctxT, routing_bc = st.vp, st.ctxT, st.routing_bc
            ph = (h % 2) * 64
            th = h // 2
            ps_ctx = psum.tile([VW, S], f32, name="ps_ctx", tag="ctx", bufs=CTX_BUFS)
            mms = [(l, r, ps_ctx[:, q0:q0 + n]) for (l, r, (q0, n)) in ctx_mms]
            for kb in range(SB):
                if qs[kb] != 0:
                    mms.append((vp[:, kb, h, :], ones_bf[:], ps_ctx[:, 0:1]))
            for i, (lhsT, rhs, dst) in enumerate(mms):
                nc.tensor.matmul(
                    dst, lhsT, rhs,
                    start=(i == 0), stop=(i == len(mms) - 1),
                    skip_group_check=True,
                )

            if ONES64:
                rec = small.tile([64, S], f32, name="recip", tag="recip")
                nc.vector.reciprocal(rec[:], ps_ctx[DK:VW, :])
                bc_ap = rec[:]
            else:
                recip = small.tile([1, S], f32, name="recip", tag="recip")
                nc.vector.reciprocal(recip[:], ps_ctx[DK:DK + 1, :])
                bc = small.tile([64, S], f32, name="bc", tag="bc", bufs=2)
                nc.gpsimd.partition_broadcast(bc[:], recip[:], channels=64)
                bc_ap = bc[:]
            nc.vector.scalar_tensor_tensor(
                ctxT[ph:ph + 64, th, :],
                ps_ctx[0:DK, :],
                routing_bc[0:64, h:h + 1],
                bc_ap,
                op0=ALU.mult, op1=ALU.mult,
            )

        def out_chunks(i):
            st = states[i]
            chunks = []

            def o_chunk(sb, c):
                def go():
                    ps = psum.tile([128, S], f32, name="mmo_ps", tag="mm",
                                   bufs=MM_BUFS)
                    for t in range(DT):
                        nc.tensor.matmul(
                            ps[:],
                            st.ctxT[:, t, sb * 128:(sb + 1) * 128],
                            w_tiles["wo"][t][:, c * 512:(c + 1) * 512],
                            start=(t == 0),
                            stop=(t == DT - 1),
                        )
                    ob = small.tile([128, S], f32, name="ob", tag="ob", bufs=2)
                    if has_bvo:
                        nc.vector.scalar_tensor_tensor(
                            ob[:], ps[:], 1.0, bob_sb[:, c * 512:(c + 1) * 512],
                            op0=ALU.mult, op1=ALU.add,
                        )
                    else:
                        nc.scalar.copy(ob[:], ps[:])
                    nc.sync.dma_start(
                        out[st.b, sb * 128:(sb + 1) * 128,
                            c * 512:(c + 1) * 512],
                        ob[:],
                    )
                return go

            for sb in range(SB):
                for c in range(2):
                    chunks.append(o_chunk(sb, c))
            return chunks

        # ---- software-pipelined schedule ------------------------------
        order = [bb for _ in range(repeat) for bb in range(b_loc)]
        N = len(order)

        load(0, order[0])
        pc0 = proj_chunks(0)
        for c in pc0:
            c()
        if N > 1:
            load(1, order[1])
        routing(0)

        for i in range(N):
            st = states[i]
            st.ctxT = act.tile([128, DT, S], bf16, name="ctxT", tag="ctxT")
            work = []
            if i + 1 < N:
                pc = proj_chunks(i + 1)
                work += pc[:DT]                      # qpT chunks first
                work.append(lambda j=i + 1: routing(j))
                work += pc[DT:]                      # kpT + vp chunks
            if i + 2 < N:
                work.insert(min(2, len(work)),
                            lambda j=i + 2: load(j, order[j]))
            if i >= 1:
                work += out_chunks(i - 1)
            L = len(work)
            done = 0
            pend = None           # (h, ctx_mms) awaiting ctx issue
            for h in range(H):
                mms = attn_scores(st, h)
                if not HPIPE:
                    attn_ctx(st, h, mms)
                else:
                    if pend is not None:
                        attn_ctx(st, pend[0], pend[1])
                    pend = (h, mms)
                want = ((h + 1) * L) // H
                while done < want:
                    work[done]()
                    done += 1
            if pend is not None:
                attn_ctx(st, pend[0], pend[1])
            while done < L:
                work[done]()
                done += 1
        for c in out_chunks(N - 1):
            c()

    nc.compile()
    return nc


def _w8(W):
    """Host prep of a [D, D] f32 weight for DoubleRow fp8: scale by 64 (the
    0.02-scale weights would land in e4m3's subnormal range; the 1/64 is
    applied at projection evacuation) and fold row pairs (t2, i):
    out[t2*128+p, i, c] = 64*W[t2*256 + i*128 + p, c]."""
    e4 = ml_dtypes.float8_e4m3
    W8 = (np.asarray(W, np.float32) * 64).reshape(DT // 2, 2, 128, D)
    return np.ascontiguousarray(W8.transpose(0, 2, 1, 3).reshape(D // 2, 2, D)).astype(e4)


def _prep_shared(Wq, Wk, Wv, Wo, Wg, bq, bk):
    bf = ml_dtypes.bfloat16
    return {
        "wq8": _w8(Wq), "wk": np.asarray(Wk).astype(bf),
        "wv": np.asarray(Wv).astype(bf), "wo": np.asarray(Wo).astype(bf),
        "wg": np.asarray(Wg).astype(bf),
        "bqt": np.ascontiguousarray(
            np.asarray(bq).astype(np.float32).reshape(DT, 128).T),
        "bkt": np.ascontiguousarray(
            np.asarray(bk).astype(np.float32).reshape(DT, 128).T),
    }


def _prep_qkv(q, k, v):
    e4 = ml_dtypes.float8_e4m3
    bf = ml_dtypes.bfloat16
    qT8 = np.ascontiguousarray(np.asarray(q).transpose(0, 2, 1)).astype(e4)
    kT = np.ascontiguousarray(np.asarray(k).astype(bf).transpose(0, 2, 1))
    vT = np.ascontiguousarray(np.asarray(v).astype(bf).transpose(0, 2, 1))
    return qT8, kT, vT


def kernel(**inputs):
    q = np.asarray(inputs["q"])
    k = np.asarray(inputs["k"])
    v = np.asarray(inputs["v"])
    mask = np.asarray(inputs["mask"]).reshape(S, S)
    Wq, bq = np.asarray(inputs["Wq"]), np.asarray(inputs["bq"])
    Wk, bk = np.asarray(inputs["Wk"]), np.asarray(inputs["bk"])
    Wv, bv = np.asarray(inputs["Wv"]), np.asarray(inputs["bv"])
    Wg = np.asarray(inputs["Wg"])
    Wo, bo = np.asarray(inputs["Wo"]), np.asarray(inputs["bo"])

    bf = ml_dtypes.bfloat16
    qs, mixed, uniq, causal = _classify_mask(mask)
    has_bvo = bool(np.any(bv) or np.any(bo))
    cache_key = ("v5", mask.tobytes(), has_bvo)
    if cache_key not in _CACHE:
        _CACHE[cache_key] = _build(qs, mixed, len(uniq), causal, has_bvo=has_bvo)
    nc = _CACHE[cache_key]

    shared = dict(_prep_shared(Wq, Wk, Wv, Wo, Wg, bq, bk))
    if has_bvo:
        shared["bvb"] = bv.astype(np.float32).reshape(1, D)
        shared["bob"] = bo.astype(np.float32).reshape(1, D)
    if uniq:
        shared["maskt"] = np.stack(uniq, axis=0)

    qT8, kT, vT = _prep_qkv(q, k, v)
    in_maps = []
    for c in range(N_CORES):
        sl = slice(c * B_LOC, (c + 1) * B_LOC)
        m = dict(shared)
        m["qT8"] = qT8[sl]
        m["kT"] = kT[sl]
        m["vT"] = vT[sl]
        in_maps.append(m)

    from concourse.bass_utils import run_bass_kernel_spmd

    kw = {}
    if PROFILE:
        import tempfile
        kw = dict(trace=True, tmpdir=tempfile.mkdtemp(prefix="moh_trace_"))
    res = None
    last_exc = None
    for _attempt in range(3):
        try:
            res = run_bass_kernel_spmd(
                nc, in_maps, core_ids=list(range(N_CORES)), **kw)
            break
        except Exception as e:  # transient axon/NRT device errors
            last_exc = e
    if res is None:
        raise last_exc
    LAST["exec_time_ns"] = res.exec_time_ns
    LAST["profile_json"] = res.profile_json
    if PROFILE:
        LAST["tmpdir"] = kw.get("tmpdir")
    outs = [res.results[c]["out"] for c in range(N_CORES)]
    return np.concatenate(outs, axis=0).astype(np.float32)


# revision 37
# speedup vs baseline: 1.2001x; 1.2001x over previous
"""MoH (Mixture-of-Heads) attention kernel for Trainium2, 8 NeuronCores.

Strategy: data-parallel over batch (32 batches -> 4 per core), weights
replicated, no collectives.  Matmuls in bf16 (fp32 PSUM accumulation)
except the q projection, which runs in fp8e4 with DoubleRow perf mode
(2x128 contraction per instruction -> ~2x faster streaming).  fp8 on the
k side too would push the score noise over the 2e-2 budget (measured
2.33e-2 both / 1.83e-2 q-only / 0.41e-2 bf16), and v/out errors flow
linearly to the output, so those stay bf16.

Layouts (per core):
  - host pre-transposes q/k/v to [B, D, S] and casts to bf16, so the
    projections need no on-device transposes:
       qpT[d',s] = sum_d Wq[d,d'] qT[d,s]   (lhsT = Wq as stored)
       vp[s,d']  = sum_d vT[d,s] Wv[d,d']   (lhsT = vT)
  - heads live in the partition dim of qpT/kpT (64 rows each), so
    transposed scores ST[k,q] = kh @ qh^T come straight from matmuls of
    qpT/kpT slices; softmax runs without max-subtraction (scores are
    O(1)); fully-masked k-blocks are skipped.
  - causal masking: post-exp multiply of each diagonal 128x128 block by
    a lower-triangular 0/1 bf16 tile built on-device with affine_select
    (general masks fall back to additive -1e9 tiles pre-exp).
  - the reference's "scores row 0 := 0" rule becomes "STexp[:,0] := 1"
    plus ones-matmuls for the skipped k-blocks.
  - vp carries a ones-column so the attention matmul also produces the
    softmax denominator (row 64 of the [65, q] PSUM tile).
  - routing: gates = softmax(qpT^T @ Wg), hard top-2 of 12 via two
    reduce_max passes, mean over S via a ones-vector matmul; the
    per-(batch,head) routing scalar and 1/denominator are fused into the
    single DVE op that moves ctx^T from PSUM to SBUF.
  - out = ctxT^T @ Wo from the same partition-sliced ctxT tiles.

Scheduling: batches flow through a software pipeline.  During batch i's
16-head attention loop (ACT-bound: one exp per score pack), the PE-heavy
projection chunks of batch i+1 and the output-projection chunks of batch
i-1 are issue-interleaved so the Tensor engine never starves; within the
head loop, head h+1's score matmuls issue before head h's ctx matmuls so
the exp latency hides behind PE work.  Input and output DMAs issue from
SP, early weights from ACT, late weights from GPSIMD, keeping the
compute engines' queues clean at startup.
"""

import sys

_TRN_REPO = "/opt/trn_rl_repo"
if _TRN_REPO not in sys.path:
    sys.path.insert(0, _TRN_REPO)

import numpy as np
import ml_dtypes

B, S, D = 32, 512, 1024
H, DK = 16, 64
H_SH, K_SEL = 4, 2
H_DYN = H - H_SH
N_CORES = 8
B_LOC = B // N_CORES
SB = S // 128      # 4 s-blocks
DT = D // 128      # 8 d-tiles
NEG = -1e9

_CACHE = {}
PROFILE = False          # set by test harness to capture an NTFF trace
LAST = {}                # exec_time_ns / profile path from the last run


def _classify_mask(mask):
    """Host-side: derive block structure from the [S,S] 0/1 mask.

    Returns (qs, mixed, uniq_tiles, causal) where
      qs[kb]    = first q (multiple of 128) kept for k-block kb, or None
      mixed[(qb,kb)] = index into uniq_tiles for blocks needing an
                  additive mask tile (maskT layout [k_local, q_local]),
                  or -1 for all-masked blocks inside the computed range
      uniq_tiles = list of [128,128] f32 additive tiles
      causal    = True if mask is exactly lower-triangular (fast path:
                  post-exp multiplicative triangular masking)
    """
    m = mask.astype(bool)
    if not m[1:].any(axis=1).all():
        raise NotImplementedError(
            "a query row (>0) is fully masked; uniform-softmax fallback "
            "for fully-masked rows is not implemented"
        )
    causal = bool(np.array_equal(m, np.tril(np.ones_like(m))))
    qs = []
    mixed = {}
    uniq = []
    uniq_key = {}
    for kb in range(SB):
        first = None
        for qb in range(SB):
            blk = m[qb * 128:(qb + 1) * 128, kb * 128:(kb + 1) * 128]
            if blk.any():
                if first is None:
                    first = qb * 128
                if not blk.all():
                    add = np.where(blk.T, 0.0, np.float32(NEG)).astype(np.float32)
                    key = add.tobytes()
                    if key not in uniq_key:
                        uniq_key[key] = len(uniq)
                        uniq.append(add)
                    mixed[(qb, kb)] = uniq_key[key]
            elif first is not None:
                mixed[(qb, kb)] = -1
        qs.append(first)
    return qs, mixed, uniq, causal


def _build(qs, mixed, uniq_n, causal, b_loc=B_LOC, has_bvo=False, repeat=1):
    import os
    PACK = int(os.environ.get("K_PACK", "1"))
    MM_BUFS = int(os.environ.get("K_MM_BUFS", "4"))
    CTX_BUFS = int(os.environ.get("K_CTX_BUFS", "2"))
    STEXP_BUFS = int(os.environ.get("K_STEXP_BUFS", "8"))
    IN_BUFS = int(os.environ.get("K_IN_BUFS", "2"))
    TRIMASK_POOL = int(os.environ.get("K_TRIMASK_POOL", "0"))
    HPIPE = int(os.environ.get("K_HPIPE", "1"))
    # K_ONES64=1 replicates the denominator across PSUM partitions 64..127
    # via 64 ones-columns in vp, removing the per-head partition_broadcast.
    # Measured SLOWER on hardware (+138us/iter): the wider stationary tile
    # makes every ctx ldweights load 128 columns instead of 65 -- weight
    # loads are not free on HW even though the cost model says they are.
    ONES64 = int(os.environ.get("K_ONES64", "0"))
    VW = 128 if ONES64 else DK + 1    # vp row width per head
    import concourse.bacc as bacc
    import concourse.tile as tile
    import concourse.mybir as mybir
    from contextlib import ExitStack

    f32 = mybir.dt.float32
    bf16 = mybir.dt.bfloat16
    fp8 = mybir.dt.float8e4
    DRow = mybir.MatmulPerfMode.DoubleRow
    AF = mybir.ActivationFunctionType
    ALU = mybir.AluOpType

    nc = bacc.Bacc(trn_type="TRN2", target_bir_lowering=False, debug=False)

    # q input + weight in fp8e4 (weight pre-scaled by 64 on host; the 1/64
    # is folded into the projection-evacuation scale).  DoubleRow matmuls
    # contract 2x128 per instruction -> half the PE streaming time.  The
    # k side stays bf16: fp8 on both sides of the score product doubles
    # the score noise and breaches the 2e-2 budget (measured 2.33e-2 vs
    # 1.82e-2 with q only).
    qT = nc.dram_tensor("qT8", [b_loc, D, S], fp8, kind="ExternalInput").ap()
    kT = nc.dram_tensor("kT", [b_loc, D, S], bf16, kind="ExternalInput").ap()
    vT = nc.dram_tensor("vT", [b_loc, D, S], bf16, kind="ExternalInput").ap()
    wq = nc.dram_tensor("wq8", [D // 2, 2, D], fp8, kind="ExternalInput").ap()
    wk = nc.dram_tensor("wk", [D, D], bf16, kind="ExternalInput").ap()
    wv = nc.dram_tensor("wv", [D, D], bf16, kind="ExternalInput").ap()
    wo = nc.dram_tensor("wo", [D, D], bf16, kind="ExternalInput").ap()
    wg = nc.dram_tensor("wg", [D, H_DYN], bf16, kind="ExternalInput").ap()
    bqt = nc.dram_tensor("bqt", [128, DT], f32, kind="ExternalInput").ap()
    bkt = nc.dram_tensor("bkt", [128, DT], f32, kind="ExternalInput").ap()
    if has_bvo:
        bvb = nc.dram_tensor("bvb", [1, D], f32, kind="ExternalInput").ap()
        bob = nc.dram_tensor("bob", [1, D], f32, kind="ExternalInput").ap()
    if uniq_n:
        maskt = nc.dram_tensor(
            "maskt", [uniq_n, 128, 128], f32, kind="ExternalInput"
        ).ap()
    out = nc.dram_tensor("out", [b_loc, S, D], f32, kind="ExternalOutput").ap()

    # pack score k-blocks into shared PSUM tiles (<=512 f32 per bank)
    packs = []   # list of list[(kb, q0, n, off)]
    if PACK:
        cur, used = [], 0
        for kb in range(SB):
            if qs[kb] is None:
                continue
            n = S - qs[kb]
            if used + n > 512 and cur:
                packs.append(cur)
                cur, used = [], 0
            cur.append((kb, qs[kb], n, used))
            used += n
        if cur:
            packs.append(cur)
    else:
        for kb in range(SB):
            if qs[kb] is None:
                continue
            packs.append([(kb, qs[kb], S - qs[kb], 0)])

    with tile.TileContext(nc) as tc, ExitStack() as ctx:
        const = ctx.enter_context(tc.tile_pool(name="const", bufs=1))
        act = ctx.enter_context(tc.tile_pool(name="act", bufs=2))
        small = ctx.enter_context(tc.tile_pool(name="small", bufs=2))
        psum = ctx.enter_context(tc.tile_pool(name="psum", bufs=1, space="PSUM"))

        states = {}

        class St:
            pass

        def load(i, b):
            st = states.setdefault(i, St())
            st.b = b
            st.ins = {}
            for nm, ap, dt8 in (("q", qT, True), ("k", kT, False),
                                ("v", vT, False)):
                t = act.tile([128, DT, S], fp8 if dt8 else bf16,
                             name=f"in_{nm}", tag=f"in_{nm}", bufs=IN_BUFS)
                for d in range(DT):
                    nc.sync.dma_start(t[:, d, :], ap[b, d * 128:(d + 1) * 128, :])
                st.ins[nm] = t
            return st

        # ---- constants: wq + biases on ACT's queue (needed first), the
        # rest on GPSIMD's queue so ACT can start qpT evacuation early ----
        ones_bf = const.tile([128, 1], bf16, name="ones_bf", tag="ones_bf")
        nc.vector.memset(ones_bf[:], 1.0)
        ones_f32 = const.tile([128, 1], f32, name="ones_f32", tag="ones_f32")
        nc.vector.memset(ones_f32[:], 1.0)
        # lower-triangular (k <= q) 0/1 multiplicative mask in [k, q] layout
        trib = const.tile([128, 128], bf16, name="trib", tag="trib")
        if causal:
            nc.gpsimd.memset(trib[:], 1.0)
            nc.gpsimd.affine_select(
                out=trib[:], in_=trib[:],
                compare_op=mybir.AluOpType.is_ge, fill=0.0,
                base=0, pattern=[[1, 128]], channel_multiplier=-1,
            )

        bq_sb = const.tile([128, DT], f32, name="bq_sb", tag="bq_sb")
        nc.scalar.dma_start(bq_sb[:], bqt[:])
        bk_sb = const.tile([128, DT], f32, name="bk_sb", tag="bk_sb")
        nc.scalar.dma_start(bk_sb[:], bkt[:])
        w_tiles = {}
        for t2 in range(DT // 2):
            t = const.tile([128, 2, D], fp8, name=f"wq8{t2}", tag=f"wq8{t2}")
            nc.scalar.dma_start(t[:], wq[t2 * 128:(t2 + 1) * 128, :, :])
            w_tiles.setdefault("wq8", []).append(t)
        for wname, wap, eng in (("wk", wk, nc.gpsimd), ("wv", wv, nc.gpsimd),
                                ("wo", wo, nc.gpsimd)):
            tl = []
            for d in range(DT):
                t = const.tile([128, D], bf16, name=f"{wname}{d}", tag=f"{wname}{d}")
                eng.dma_start(t[:], wap[d * 128:(d + 1) * 128, :])
                tl.append(t)
            w_tiles[wname] = tl
        wg_tiles = []
        for d in range(DT):
            t = const.tile([128, H_DYN], bf16, name=f"wg{d}", tag=f"wg{d}")
            nc.gpsimd.dma_start(t[:], wg[d * 128:(d + 1) * 128, :])
            wg_tiles.append(t)
        if has_bvo:
            bv_sb = const.tile([1, D], f32, name="bv_sb", tag="bv_sb")
            nc.gpsimd.dma_start(bv_sb[:], bvb[:])
            bo_sb = const.tile([1, D], f32, name="bo_sb", tag="bo_sb")
            nc.gpsimd.dma_start(bo_sb[:], bob[:])
            bvb_sb = const.tile([128, D], f32, name="bvb_sb", tag="bvb_sb")
            nc.gpsimd.partition_broadcast(bvb_sb[:], bv_sb[:])
            bob_sb = const.tile([128, D], f32, name="bob_sb", tag="bob_sb")
            nc.gpsimd.partition_broadcast(bob_sb[:], bo_sb[:])

        mask_tiles = []
        for u in range(uniq_n):
            t = const.tile([128, 128], f32, name=f"mask{u}", tag=f"mask{u}")
            if not causal:
                nc.gpsimd.dma_start(t[:], maskt[u])
            mask_tiles.append(t)

        # ---------------------------------------------------------------
        def proj_chunks(i):
            """Allocate batch i's projection outputs; return issue closures."""
            st = states[i]
            st.qpT = act.tile([128, DT, S], bf16, name="qpT", tag="qpT")
            st.kpT = act.tile([128, DT, S], bf16, name="kpT", tag="kpT")
            st.vp = act.tile([128, SB, H, VW], bf16, name="vp", tag="vp")
            chunks = []

            def q_chunk(t):
                def go():
                    ps = psum.tile([128, S], f32, name="mm_ps", tag="mm",
                                   bufs=MM_BUFS)
                    src4 = st.ins["q"][:].rearrange("p (t2 i) s -> p t2 i s", i=2)
                    for t2 in range(DT // 2):
                        nc.tensor.matmul(
                            ps[:],
                            w_tiles["wq8"][t2][:, :, t * 128:(t + 1) * 128],
                            src4[:, t2, :, :],
                            start=(t2 == 0),
                            stop=(t2 == DT // 2 - 1),
                            perf_mode=DRow,
                        )
                    nc.scalar.activation(
                        st.qpT[:, t, :], ps[:], AF.Identity,
                        bias=bq_sb[:, t:t + 1], scale=1.0 / 64,
                    )
                return go

            def k_chunk(t):
                def go():
                    ps = psum.tile([128, S], f32, name="mm_ps", tag="mm",
                                   bufs=MM_BUFS)
                    for d in range(DT):
                        nc.tensor.matmul(
                            ps[:],
                            w_tiles["wk"][d][:, t * 128:(t + 1) * 128],
                            st.ins["k"][:, d, :],
                            start=(d == 0),
                            stop=(d == DT - 1),
                        )
                    nc.vector.tensor_scalar_add(
                        st.kpT[:, t, :], ps[:], bk_sb[:, t:t + 1]
                    )
                return go

            for t in range(DT):
                chunks.append(q_chunk(t))
            for t in range(DT):
                chunks.append(k_chunk(t))

            def v_chunk(sb, c):
                def go():
                    if sb == 0 and c == 0 and (not ONES64 or i < 2):
                        # ONES64: the ones region is never overwritten, and
                        # the vp tag rotates 2 buffers, so batches 0/1 cover
                        # all buffers and later batches skip the memset
                        nc.vector.memset(st.vp[:, :, :, DK:VW], 1.0)
                    ps = psum.tile([128, S], f32, name="mmv_ps", tag="mm",
                                   bufs=MM_BUFS)
                    for d in range(DT):
                        nc.tensor.matmul(
                            ps[:],
                            st.ins["v"][:, d, sb * 128:(sb + 1) * 128],
                            w_tiles["wv"][d][:, c * 512:(c + 1) * 512],
                            start=(d == 0),
                            stop=(d == DT - 1),
                        )
                    src2 = ps[:].rearrange("p (h e) -> p h e", e=DK)
                    dst2 = st.vp[:, sb, c * 8:(c + 1) * 8, 0:DK]
                    if has_bvo:
                        nc.vector.scalar_tensor_tensor(
                            dst2, src2, 1.0,
                            bvb_sb[:, c * 512:(c + 1) * 512].rearrange(
                                "p (h e) -> p h e", e=DK),
                            op0=ALU.mult, op1=ALU.add,
                        )
                    else:
                        nc.vector.tensor_copy(dst2, src2)
                return go

            for sb in range(SB):
                for c in range(2):
                    chunks.append(v_chunk(sb, c))
            return chunks

        def routing(i):
            st = states[i]
            qpT = st.qpT
            ps_r = psum.tile([1, H_DYN], f32, name="ps_r", tag="rsum", bufs=1)
            for sb in range(SB):
                ps_g = psum.tile([128, H_DYN], f32, name="ps_g", tag="gat", bufs=1)
                for t in range(DT):
                    nc.tensor.matmul(
                        ps_g[:],
                        qpT[:, t, sb * 128:(sb + 1) * 128],
                        wg_tiles[t][:],
                        start=(t == 0),
                        stop=(t == DT - 1),
                    )
                gexp = small.tile([128, H_DYN], f32, name="gexp", tag="gexp")
                gsum = small.tile([128, 1], f32, name="gsum", tag="gsum")
                nc.scalar.activation(gexp[:], ps_g[:], AF.Exp, accum_out=gsum[:])
                ginv = small.tile([128, 1], f32, name="ginv", tag="ginv")
                nc.vector.reciprocal(ginv[:], gsum[:])
                m1 = small.tile([128, 1], f32, name="m1", tag="m1")
                nc.vector.reduce_max(m1[:], gexp[:], axis=mybir.AxisListType.X)
                eqm = small.tile([128, H_DYN], f32, name="eqm", tag="eqm")
                nc.vector.tensor_scalar(eqm[:], gexp[:], m1[:], None,
                                        op0=ALU.is_equal)
                g2 = small.tile([128, H_DYN], f32, name="g2", tag="g2")
                nc.vector.scalar_tensor_tensor(
                    g2[:], eqm[:], NEG, gexp[:], op0=ALU.mult, op1=ALU.add
                )
                m2 = small.tile([128, 1], f32, name="m2", tag="m2")
                nc.vector.reduce_max(m2[:], g2[:], axis=mybir.AxisListType.X)
                sel = small.tile([128, H_DYN], f32, name="sel", tag="sel")
                nc.vector.tensor_scalar(sel[:], gexp[:], m2[:], None,
                                        op0=ALU.is_ge)
                dyn = small.tile([128, H_DYN], f32, name="dyn", tag="dyn")
                nc.vector.tensor_scalar(dyn[:], gexp[:], ginv[:], None,
                                        op0=ALU.mult)
                nc.vector.tensor_tensor(dyn[:], dyn[:], sel[:], op=ALU.mult)
                nc.tensor.matmul(
                    ps_r[:], ones_f32[:], dyn[:],
                    start=(sb == 0), stop=(sb == SB - 1),
                    skip_group_check=True,
                )
            routing_sb = small.tile([1, H], f32, name="routing_sb", tag="routing_sb")
            nc.vector.memset(routing_sb[0:1, 0:H_SH], 1.0)
            nc.scalar.mul(routing_sb[0:1, H_SH:H], ps_r[0:1, :], 1.0 / S)
            routing_bc = small.tile([128, H], f32, name="routing_bc", tag="routing_bc")
            nc.gpsimd.partition_broadcast(routing_bc[:], routing_sb[:])
            st.routing_bc = routing_bc

        def attn_scores(st, h):
            """Score matmuls + exp + masking for head h; returns the se
            slices the ctx phase needs (issued separately so the next
            head's scores hide this head's exp latency on the PE)."""
            qpT, kpT = st.qpT, st.kpT
            vp = st.vp
            ph = (h % 2) * 64
            th = h // 2
            ctx_mms = []          # deferred (lhsT, rhs, (q0, n))
            for grp in packs:
                tot = grp[-1][3] + grp[-1][2]
                ps_st = psum.tile([128, 512], f32, name="ps_st", tag="mm", bufs=MM_BUFS)
                se = small.tile([128, 512], bf16, name="stexp", tag="stexp", bufs=STEXP_BUFS)
                for (kb, q0, n, off) in grp:
                    nc.tensor.matmul(
                        ps_st[:, off:off + n],
                        kpT[ph:ph + 64, th, kb * 128:(kb + 1) * 128],
                        qpT[ph:ph + 64, th, q0:S],
                        start=True, stop=True,
                        skip_group_check=True,
                    )
                    if not causal:
                        for qb in range(q0 // 128, SB):
                            mi = mixed.get((qb, kb))
                            if mi is not None and mi >= 0:
                                sl = ps_st[:, off + qb * 128 - q0:
                                           off + (qb + 1) * 128 - q0]
                                nc.vector.tensor_tensor(
                                    sl, sl, mask_tiles[mi][:], op=ALU.add
                                )
                nc.scalar.activation(
                    se[:, 0:tot], ps_st[:, 0:tot], AF.Exp, scale=1.0 / np.sqrt(DK)
                )
                if causal:
                    for (kb, q0, n, off) in grp:
                        if TRIMASK_POOL:
                            nc.gpsimd.affine_select(
                                out=se[:, off:off + 128],
                                in_=se[:, off:off + 128],
                                compare_op=ALU.is_ge, fill=0.0,
                                base=0, pattern=[[1, 128]],
                                channel_multiplier=-1,
                            )
                        else:
                            nc.vector.tensor_tensor(
                                se[:, off:off + 128], se[:, off:off + 128],
                                trib[:], op=ALU.mult,
                            )
                        if q0 == 0:
                            nc.vector.memset(se[:, off:off + 1], 1.0)
                else:
                    for (kb, q0, n, off) in grp:
                        for qb in range(q0 // 128, SB):
                            if mixed.get((qb, kb)) == -1:
                                nc.vector.memset(
                                    se[:, off + qb * 128 - q0:
                                       off + (qb + 1) * 128 - q0], 0.0)
                        if q0 == 0:
                            nc.vector.memset(se[:, off:off + 1], 1.0)
                for (kb, q0, n, off) in grp:
                    ctx_mms.append((vp[:, kb, h, :], se[:, off:off + n],
                                    (q0, n)))
            return ctx_mms

        def attn_ctx(st, h, ctx_mms):
            vp, ctxT, routing_bc = st.vp, st.ctxT, st.routing_bc
            ph = (h % 2) * 64
            th = h // 2
            ps_ctx = psum.tile([VW, S], f32, name="ps_ctx", tag="ctx", bufs=CTX_BUFS)
            mms = [(l, r, ps_ctx[:, q0:q0 + n]) for (l, r, (q0, n)) in ctx_mms]
            for kb in range(SB):
                if qs[kb] != 0:
                    mms.append((vp[:, kb, h, :], ones_bf[:], ps_ctx[:, 0:1]))
            for i, (lhsT, rhs, dst) in enumerate(mms):
                nc.tensor.matmul(
                    dst, lhsT, rhs,
                    start=(i == 0), stop=(i == len(mms) - 1),
                    skip_group_check=True,
                )

            if ONES64:
                rec = small.tile([64, S], f32, name="recip", tag="recip")
                nc.vector.reciprocal(rec[:], ps_ctx[DK:VW, :])
                bc_ap = rec[:]
            else:
                recip = small.tile([1, S], f32, name="recip", tag="recip")
                nc.vector.reciprocal(recip[:], ps_ctx[DK:DK + 1, :])
                bc = small.tile([64, S], f32, name="bc", tag="bc", bufs=2)
                nc.gpsimd.partition_broadcast(bc[:], recip[:], channels=64)
                bc_ap = bc[:]
            nc.vector.scalar_tensor_tensor(
                ctxT[ph:ph + 64, th, :],
                ps_ctx[0:DK, :],
                routing_bc[0:64, h:h + 1],
                bc_ap,
                op0=ALU.mult, op1=ALU.mult,
            )

        def out_chunks(i):
            st = states[i]
            chunks = []

            def o_chunk(sb, c):
                def go():
                    ps = psum.tile([128, S], f32, name="mmo_ps", tag="mm",
                                   bufs=MM_BUFS)
                    for t in range(DT):
                        nc.tensor.matmul(
                            ps[:],
                            st.ctxT[:, t, sb * 128:(sb + 1) * 128],
                            w_tiles["wo"][t][:, c * 512:(c + 1) * 512],
                            start=(t == 0),
                            stop=(t == DT - 1),
                        )
                    ob = small.tile([128, S], f32, name="ob", tag="ob", bufs=2)
                    if has_bvo:
                        nc.vector.scalar_tensor_tensor(
                            ob[:], ps[:], 1.0, bob_sb[:, c * 512:(c + 1) * 512],
                            op0=ALU.mult, op1=ALU.add,
                        )
                    else:
                        nc.scalar.copy(ob[:], ps[:])
                    nc.sync.dma_start(
                        out[st.b, sb * 128:(sb + 1) * 128,
                            c * 512:(c + 1) * 512],
                        ob[:],
                    )
                return go

            for sb in range(SB):
                for c in range(2):
                    chunks.append(o_chunk(sb, c))
            return chunks

        # ---- software-pipelined schedule ------------------------------
        order = [bb for _ in range(repeat) for bb in range(b_loc)]
        N = len(order)

        load(0, order[0])
        pc0 = proj_chunks(0)
        for c in pc0:
            c()
        if N > 1:
            load(1, order[1])
        routing(0)

        for i in range(N):
            st = states[i]
            st.ctxT = act.tile([128, DT, S], bf16, name="ctxT", tag="ctxT")
            work = []
            if i + 1 < N:
                pc = proj_chunks(i + 1)
                work += pc[:DT]                      # qpT chunks first
                work.append(lambda j=i + 1: routing(j))
                work += pc[DT:]                      # kpT + vp chunks
            if i + 2 < N:
                work.insert(min(2, len(work)),
                            lambda j=i + 2: load(j, order[j]))
            if i >= 1:
                work += out_chunks(i - 1)
            L = len(work)
            done = 0
            pend = None           # (h, ctx_mms) awaiting ctx issue
            for h in range(H):
                mms = attn_scores(st, h)
                if not HPIPE:
                    attn_ctx(st, h, mms)
                else:
                    if pend is not None:
                        attn_ctx(st, pend[0], pend[1])
                    pend = (h, mms)
                want = ((h + 1) * L) // H
                while done < want:
                    work[done]()
                    done += 1
            if pend is not None:
                attn_ctx(st, pend[0], pend[1])
            while done < L:
                work[done]()
                done += 1
        for c in out_chunks(N - 1):
            c()

    nc.compile()
    return nc


def _w8(W):
    """Host prep of a [D, D] f32 weight for DoubleRow fp8: scale by 64 (the
    0.02-scale weights would land in e4m3's subnormal range; the 1/64 is
    applied at projection evacuation) and fold row pairs (t2, i):
    out[t2*128+p, i, c] = 64*W[t2*256 + i*128 + p, c]."""
    e4 = ml_dtypes.float8_e4m3
    W8 = (np.asarray(W, np.float32) * 64).reshape(DT // 2, 2, 128, D)
    return np.ascontiguousarray(W8.transpose(0, 2, 1, 3).reshape(D // 2, 2, D)).astype(e4)


def _prep_shared(Wq, Wk, Wv, Wo, Wg, bq, bk):
    bf = ml_dtypes.bfloat16
    return {
        "wq8": _w8(Wq), "wk": np.asarray(Wk).astype(bf),
        "wv": np.asarray(Wv).astype(bf), "wo": np.asarray(Wo).astype(bf),
        "wg": np.asarray(Wg).astype(bf),
        "bqt": np.ascontiguousarray(
            np.asarray(bq).astype(np.float32).reshape(DT, 128).T),
        "bkt": np.ascontiguousarray(
            np.asarray(bk).astype(np.float32).reshape(DT, 128).T),
    }


def _prep_qkv(q, k, v):
    e4 = ml_dtypes.float8_e4m3
    bf = ml_dtypes.bfloat16
    qT8 = np.ascontiguousarray(np.asarray(q).transpose(0, 2, 1)).astype(e4)
    kT = np.ascontiguousarray(np.asarray(k).astype(bf).transpose(0, 2, 1))
    vT = np.ascontiguousarray(np.asarray(v).astype(bf).transpose(0, 2, 1))
    return qT8, kT, vT


def kernel(**inputs):
    q = np.asarray(inputs["q"])
    k = np.asarray(inputs["k"])
    v = np.asarray(inputs["v"])
    mask = np.asarray(inputs["mask"]).reshape(S, S)
    Wq, bq = np.asarray(inputs["Wq"]), np.asarray(inputs["bq"])
    Wk, bk = np.asarray(inputs["Wk"]), np.asarray(inputs["bk"])
    Wv, bv = np.asarray(inputs["Wv"]), np.asarray(inputs["bv"])
    Wg = np.asarray(inputs["Wg"])
    Wo, bo = np.asarray(inputs["Wo"]), np.asarray(inputs["bo"])

    bf = ml_dtypes.bfloat16
    qs, mixed, uniq, causal = _classify_mask(mask)
    has_bvo = bool(np.any(bv) or np.any(bo))
    cache_key = ("v5", mask.tobytes(), has_bvo)
    if cache_key not in _CACHE:
        _CACHE[cache_key] = _build(qs, mixed, len(uniq), causal, has_bvo=has_bvo)
    nc = _CACHE[cache_key]

    shared = dict(_prep_shared(Wq, Wk, Wv, Wo, Wg, bq, bk))
    if has_bvo:
        shared["bvb"] = bv.astype(np.float32).reshape(1, D)
        shared["bob"] = bo.astype(np.float32).reshape(1, D)
    if uniq:
        shared["maskt"] = np.stack(uniq, axis=0)

    qT8, kT, vT = _prep_qkv(q, k, v)
    in_maps = []
    for c in range(N_CORES):
        sl = slice(c * B_LOC, (c + 1) * B_LOC)
        m = dict(shared)
        m["qT8"] = qT8[sl]
        m["kT"] = kT[sl]
        m["vT"] = vT[sl]
        in_maps.append(m)

    from concourse.bass_utils import run_bass_kernel_spmd

    kw = {}
    if PROFILE:
        import tempfile
        kw = dict(trace=True, tmpdir=tempfile.mkdtemp(prefix="moh_trace_"))
    res = None
    last_exc = None
    for _attempt in range(3):
        try:
            res = run_bass_kernel_spmd(
                nc, in_maps, core_ids=list(range(N_CORES)), **kw)
            break
        except Exception as e:  # transient axon/NRT device errors
            last_exc = e
    if res is None:
        raise last_exc
    LAST["exec_time_ns"] = res.exec_time_ns
    LAST["profile_json"] = res.profile_json
    if PROFILE:
        LAST["tmpdir"] = kw.get("tmpdir")
    outs = [res.results[c]["out"] for c in range(N_CORES)]
    return np.concatenate(outs, axis=0).astype(np.float32)
